# revision 7
# baseline (speedup 1.0000x reference)
"""LIF neuron scan kernel for Trainium2 (8 NeuronCores, raw Bass SPMD).

Math (per timestep, fp32): v = v_prev*0.5 + x + r; s = (v > 0); v *= (1-s).
Reset+leak fold to v = 0.5*min(v_prev, 0) + (x + r).  With the exact fp32
rescaling w_t = 2^t * v_t (power-of-two scaling commutes with IEEE rounding)
the recurrence becomes a single fused op per step:
    w_t = min(w_{t-1}, 0) + U'_t,   U'_t = 2^t * (x_t + r_t)
and s_t = (w_t > 0) = (v_t > 0).  2^t*x, 2^t*r are prescaled on host (exact;
max |w| ~ 2^99 * 16 << fp32 max).  Spikes are emitted as uint8 via the
activation engine's Sign (u8 saturating cast maps {-1,0,1}->{0,0,1}); host
decodes (raw == 1) -> f32.

Sharding: data-parallel along batch; core i gets inp[:, 8i:8i+8, :].
Per-core DRAM layout is pre-transposed on host to [128 partitions, T, 128]
so every DMA line is contiguous per partition.

Write-visibility discipline (observed on HW): an engine's posted SBUF writes
can lag its semaphore increment by over a microsecond when DMA traffic
saturates SBUF, so a consumer on ANOTHER engine (or a DMA read) may see
stale data.  Every semaphore increment that publishes freshly written data
to another engine/DMA therefore goes through an explicit engine DRAIN
(maybe_drain_then_inc / drain before dma_start).  Same-engine chains are
safe (in-order SBUF port).
"""
import sys
sys.path.insert(0, "/opt/trn_rl_repo")
import numpy as np
import concourse.bass as bass
from concourse import mybir
from concourse.bass_utils import run_bass_kernel_spmd

F32 = mybir.dt.float32
U8 = mybir.dt.uint8
T, B, N = 100, 64, 2048
NCORES = 8
B_LOC = B // NCORES
P = 128
F = (B_LOC * N) // P      # 128 free elems per step
K = 10                    # timesteps per chunk
C = T // K
KF = K * F


def _build_nc():
    nc = bass.Bass()
    x_ext = nc.dram_tensor("x", [P, T * F], F32, kind="ExternalInput")
    r_ext = nc.dram_tensor("r", [P, T * F], F32, kind="ExternalInput")
    s_ext = nc.dram_tensor("s", [P, T * F], U8, kind="ExternalOutput")

    with (
        nc.sbuf_tensor([P, 2, KF], F32) as xb,
        nc.sbuf_tensor([P, 2, KF], F32) as rb,
        nc.sbuf_tensor([P, 2, KF], F32) as ub,
        nc.sbuf_tensor([P, 3, KF], F32) as wb,
        nc.sbuf_tensor([P, 4, KF], U8) as sb,
        nc.sbuf_tensor([P, F], F32) as z0,
        nc.semaphore() as sem_x,
        nc.semaphore() as sem_r,
        nc.semaphore() as pool_done,
        nc.semaphore() as dve_done,
        nc.semaphore() as act_done,
        nc.semaphore() as sem_out,
        nc.Block() as block,
    ):
        @block.sync
        def _(sync):
            for c in range(C):
                b = c % 2
                if c >= 2:
                    sync.wait_ge(pool_done, c - 1)   # xb/rb[b] freed by Pool(c-2)
                sync.dma_start(xb[:, b, :], x_ext[:, c*KF:(c+1)*KF]).then_inc(sem_x, 16)
                sync.dma_start(rb[:, b, :], r_ext[:, c*KF:(c+1)*KF]).then_inc(sem_r, 16)

        @block.gpsimd
        def _(pool):
            for c in range(C):
                b = c % 2
                pool.wait_ge(sem_x, 16 * (c + 1))
                pool.wait_ge(sem_r, 16 * (c + 1))
                if c >= 2:
                    pool.wait_ge(dve_done, c - 1)    # ub[b] freed by DVE(c-2)
                nc.gpsimd.tensor_tensor(ub[:, b, :], xb[:, b, :], rb[:, b, :],
                                        mybir.AluOpType.add).then_inc(pool_done, 1)

        @block.vector
        def _(vector):
            nc.vector.memset(z0[:], 0.0)
            for c in range(C):
                w = c % 3
                vector.wait_ge(pool_done, c + 1)
                if c >= 3:
                    vector.wait_ge(act_done, c - 2)  # wb[w] freed by Act(c-3)
                for k in range(K):
                    if c == 0 and k == 0:
                        wprev = z0[:]
                    elif k == 0:
                        wprev = wb[:, (c-1) % 3, (K-1)*F:]
                    else:
                        wprev = wb[:, w, (k-1)*F:k*F]
                    nc.vector.scalar_tensor_tensor(
                        wb[:, w, k*F:(k+1)*F], wprev, 0.0,
                        ub[:, c % 2, k*F:(k+1)*F],
                        mybir.AluOpType.min, mybir.AluOpType.add)
                # flush posted wb writes before publishing the chunk to Act
                vector.maybe_drain_then_inc((dve_done, 1))

        @block.scalar
        def _(act):
            for c in range(C):
                act.wait_ge(dve_done, c + 1)
                if c >= 4:
                    # sb[c%4] read by out-DMA(c-4); out-DMA(c-3)'s increments
                    # give a one-transfer margin on top.
                    act.wait_ge(sem_out, 16 * (c - 2))
                nc.scalar.activation(sb[:, c % 4, :], wb[:, c % 3, :],
                                     mybir.ActivationFunctionType.Sign).then_inc(act_done, 1)
                # flush sign's posted sb writes before the DMA engines read them
                act.drain()
                act.dma_start(s_ext[:, c*KF:(c+1)*KF], sb[:, c % 4, :]).then_inc(sem_out, 16)

    return nc


_SCALE = np.exp2(np.arange(T, dtype=np.float32)).astype(np.float32)


def _shard(inp: np.ndarray, rec: np.ndarray) -> list[dict[str, np.ndarray]]:
    # prescale by 2^t (exact in fp32), then per-core transpose to [P, T*F]
    xs_all = inp.reshape(T, B, N) * _SCALE[:, None, None]
    rs_all = rec.reshape(T, B, N) * _SCALE[:, None, None]
    in_maps = []
    for i in range(NCORES):
        xs = xs_all[:, i*B_LOC:(i+1)*B_LOC, :].reshape(T, P, F)
        rs = rs_all[:, i*B_LOC:(i+1)*B_LOC, :].reshape(T, P, F)
        in_maps.append({
            "x": np.ascontiguousarray(xs.transpose(1, 0, 2)).reshape(P, T * F),
            "r": np.ascontiguousarray(rs.transpose(1, 0, 2)).reshape(P, T * F),
        })
    return in_maps


def kernel(inp: np.ndarray, rec: np.ndarray) -> np.ndarray:
    inp = np.asarray(inp, dtype=np.float32)
    rec = np.asarray(rec, dtype=np.float32)
    nc = _build_nc()
    in_maps = _shard(inp, rec)
    res = run_bass_kernel_spmd(nc, in_maps, list(range(NCORES)))
    outs = []
    for i in range(NCORES):
        raw = res.results[i]["s"].reshape(P, T, F)          # uint8
        s = (raw == 1).astype(np.float32).transpose(1, 0, 2)  # [T, P, F]
        outs.append(s.reshape(T, B_LOC, N))
    return np.concatenate(outs, axis=1)


# revision 8
# speedup vs baseline: 1.0397x; 1.0397x over previous
"""LIF neuron scan kernel for Trainium2 (8 NeuronCores, raw Bass SPMD).

Math (per timestep, fp32): v = v_prev*0.5 + x + r; s = (v > 0); v *= (1-s).
Reset+leak fold to v = 0.5*min(v_prev, 0) + (x + r).  With the exact fp32
rescaling w_t = 2^t * v_t (power-of-two scaling commutes with IEEE rounding)
the recurrence becomes a single fused op per step:
    w_t = min(w_{t-1}, 0) + U'_t,   U'_t = 2^t * (x_t + r_t)
and s_t = (w_t > 0) = (v_t > 0).  2^t*x, 2^t*r are prescaled on host (exact;
max |w| ~ 2^99 * 16 << fp32 max).  Spikes are emitted as uint8 via the
activation engine's Sign (u8 saturating cast maps {-1,0,1}->{0,0,1}); host
decodes (raw == 1) -> f32.

Sharding: data-parallel along batch; core i gets inp[:, 8i:8i+8, :].
Per-core DRAM layout is pre-transposed on host to [128 partitions, T, 128]
so every DMA line is contiguous per partition.  Warm-up chunks (4, 6 steps)
collapse the pipeline ramp before the steady 10-step chunks.

Write-visibility discipline (observed on HW): an engine's posted SBUF writes
can lag its semaphore increment by over a microsecond under DMA pressure, so
a consumer on another engine (or a DMA read) may see stale data.  The DVE
chain publishes chunk c to Act only after DVE retires chunk c+1 (a full
chunk of settling time, off the critical path; a trailing DRAIN covers the
last chunk).  Act DRAINs its Sign writes before the out-DMA reads them.
GpSimd ops are auto-drained by the framework before their increments.
"""
import sys
sys.path.insert(0, "/opt/trn_rl_repo")
import numpy as np
import concourse.bass as bass
from concourse import mybir
from concourse.bass_utils import run_bass_kernel_spmd

F32 = mybir.dt.float32
U8 = mybir.dt.uint8
T, B, N = 100, 64, 2048
NCORES = 8
B_LOC = B // NCORES
P = 128
F = (B_LOC * N) // P      # 128 free elems per step
SIZES = [4, 6] + [10] * 9  # steps per chunk (warm-up ramp), sums to T
OFFS = [sum(SIZES[:i]) for i in range(len(SIZES))]
C = len(SIZES)
KF = max(SIZES) * F


def _build_nc():
    nc = bass.Bass()
    x_ext = nc.dram_tensor("x", [P, T * F], F32, kind="ExternalInput")
    r_ext = nc.dram_tensor("r", [P, T * F], F32, kind="ExternalInput")
    s_ext = nc.dram_tensor("s", [P, T * F], U8, kind="ExternalOutput")

    with (
        nc.sbuf_tensor([P, 2, KF], F32) as xb,
        nc.sbuf_tensor([P, 2, KF], F32) as rb,
        nc.sbuf_tensor([P, 2, KF], F32) as ub,
        nc.sbuf_tensor([P, 4, KF], F32) as wb,
        nc.sbuf_tensor([P, 4, KF], U8) as sb,
        nc.sbuf_tensor([P, F], F32) as z0,
        nc.semaphore() as sem_x,
        nc.semaphore() as sem_r,
        nc.semaphore() as pool_done,
        nc.semaphore() as dve_done,
        nc.semaphore() as act_done,
        nc.semaphore() as sem_out,
        nc.Block() as block,
    ):
        @block.sync
        def _(sync):
            for c in range(C):
                b = c % 2
                lo, kf = OFFS[c] * F, SIZES[c] * F
                if c >= 2:
                    sync.wait_ge(pool_done, c - 1)   # xb/rb[b] freed by Pool(c-2)
                sync.dma_start(xb[:, b, :kf], x_ext[:, lo:lo+kf]).then_inc(sem_x, 16)
                sync.dma_start(rb[:, b, :kf], r_ext[:, lo:lo+kf]).then_inc(sem_r, 16)

        @block.gpsimd
        def _(pool):
            for c in range(C):
                b = c % 2
                kf = SIZES[c] * F
                pool.wait_ge(sem_x, 16 * (c + 1))
                pool.wait_ge(sem_r, 16 * (c + 1))
                if c >= 2:
                    pool.wait_ge(dve_done, c - 1)    # ub[b] freed by DVE(c-2)
                nc.gpsimd.tensor_tensor(ub[:, b, :kf], xb[:, b, :kf], rb[:, b, :kf],
                                        mybir.AluOpType.add).then_inc(pool_done, 1)

        @block.vector
        def _(vector):
            nc.vector.memset(z0[:], 0.0)
            for c in range(C):
                w = c % 4
                vector.wait_ge(pool_done, c + 1)
                if c >= 4:
                    vector.wait_ge(act_done, c - 3)  # wb[w] freed by Act(c-4)
                for k in range(SIZES[c]):
                    if c == 0 and k == 0:
                        wprev = z0[:]
                    elif k == 0:
                        wprev = wb[:, (c-1) % 4, (SIZES[c-1]-1)*F:SIZES[c-1]*F]
                    else:
                        wprev = wb[:, w, (k-1)*F:k*F]
                    ins = nc.vector.scalar_tensor_tensor(
                        wb[:, w, k*F:(k+1)*F], wprev, 0.0,
                        ub[:, c % 2, k*F:(k+1)*F],
                        mybir.AluOpType.min, mybir.AluOpType.add)
                    if k == SIZES[c] - 1:
                        ins.then_inc(dve_done, 1)
            # final publish: flush the last chunk's posted writes
            vector.maybe_drain_then_inc((dve_done, 1))

        @block.scalar
        def _(act):
            for c in range(C):
                lo, kf = OFFS[c] * F, SIZES[c] * F
                # one-chunk lag: chunk c's wb writes settle while DVE runs
                # chunk c+1 (the final drain-inc covers c = C-1).
                act.wait_ge(dve_done, c + 2)
                if c >= 4:
                    # sb[c%4] read by out-DMA(c-4); out-DMA(c-3)'s increments
                    # give a one-transfer margin on top.
                    act.wait_ge(sem_out, 16 * (c - 2))
                nc.scalar.activation(sb[:, c % 4, :kf], wb[:, c % 4, :kf],
                                     mybir.ActivationFunctionType.Sign).then_inc(act_done, 1)
                # flush sign's posted sb writes before the DMA engines read them
                act.drain()
                act.dma_start(s_ext[:, lo:lo+kf], sb[:, c % 4, :kf]).then_inc(sem_out, 16)

    return nc


_SCALE = np.exp2(np.arange(T, dtype=np.float32)).astype(np.float32)


def _shard(inp: np.ndarray, rec: np.ndarray) -> list[dict[str, np.ndarray]]:
    # prescale by 2^t (exact in fp32), then per-core transpose to [P, T*F]
    xs_all = inp.reshape(T, B, N) * _SCALE[:, None, None]
    rs_all = rec.reshape(T, B, N) * _SCALE[:, None, None]
    in_maps = []
    for i in range(NCORES):
        xs = xs_all[:, i*B_LOC:(i+1)*B_LOC, :].reshape(T, P, F)
        rs = rs_all[:, i*B_LOC:(i+1)*B_LOC, :].reshape(T, P, F)
        in_maps.append({
            "x": np.ascontiguousarray(xs.transpose(1, 0, 2)).reshape(P, T * F),
            "r": np.ascontiguousarray(rs.transpose(1, 0, 2)).reshape(P, T * F),
        })
    return in_maps


def kernel(inp: np.ndarray, rec: np.ndarray) -> np.ndarray:
    inp = np.asarray(inp, dtype=np.float32)
    rec = np.asarray(rec, dtype=np.float32)
    nc = _build_nc()
    in_maps = _shard(inp, rec)
    res = run_bass_kernel_spmd(nc, in_maps, list(range(NCORES)))
    outs = []
    for i in range(NCORES):
        raw = res.results[i]["s"].reshape(P, T, F)          # uint8
        s = (raw == 1).astype(np.float32).transpose(1, 0, 2)  # [T, P, F]
        outs.append(s.reshape(T, B_LOC, N))
    return np.concatenate(outs, axis=1)
